# revision 17
# baseline (speedup 1.0000x reference)
"""Trainium2 Bass kernel for the BasicRNN problem.

Math: the reference's 9 block matmuls per step collapse to
    state_{t+1} = relu(state_t @ W + gate_t * [E, 0, 0]),  state [256, 4096]
with E = x @ in_w.T + in_b, gates at t % 5 == 0, output = O_T @ out_w.T + out_b
where O is the last 1024 state columns.

Device strategy (8 cores, tensor-parallel over W columns):
- Keep the state transposed on-chip: sT = state.T [4096, 256], stored as 32
  row-chunks of 128. W stays stationary on the PE; each step streams sT
  through it: next_sT[rows r] = W[:, r].T @ sT.
- The 4096 state rows are block-permuted so core c owns device blocks
  4c..4c+3 = original chunks {S_c, I_2c, I_2c+1, O_c}. Every core owns one
  S chunk and one O chunk (balanced last step, which only needs O).
- Matmul operands are bf16 (fp32 matmul is 4x slower); PSUM accumulates fp32.
- The E projection (x @ in_w.T) is computed FULLY REPLICATED on every core
  (128 extra MMs, ~14us) so state_1 = [relu(E),0,0] needs no collective at
  all and the ~47us cold communicator setup (kicked off by a dummy 1-column
  AllGather issued first) is hidden under E + round-1 compute.
- Steady rounds ship the new state in two collectives: AG_a = block 0
  (64KB/rank -> Mesh ~8us) then AG_b = blocks 1-3 (192KB/rank -> RDH ~16us).
  The CC stream executes collectives strictly serially, so the round period
  is ~= m0-tail + AG_a + AG_b ~= 28-31us; everything else (loads on 4 DGE
  queues, phase-A pre-run of the NEXT round during AG_b flight, fillers to
  keep the HAM clock gate at 2.4GHz) hides under that.
- Per round the PE does: phase A = 8 AG_a chunks x 4 m-blocks (k-outer,
  4 accumulating PSUMs), FILL_A fillers (pinned by a data dep so the
  scheduler can't float them), phase B = 24 AG_b chunks with m0's
  contraction completed first so AG_a(r) can launch ~8us before AG_b(r).
- Final classifier is sharded: core c computes out.T rows [125c, 125c+125)
  with bias; host concatenates and transposes.
"""

import numpy as np

S_DIM, I_DIM, O_DIM = 1024, 2048, 1024
TOTAL = 4096
INPUT_DIM, NUM_CLASSES, BATCH = 2048, 1000, 256
NC = 8
KC = TOTAL // 128          # 32 k-chunks of 128
MPC = KC // NC             # 4 blocks per core
CLS_M = NUM_CLASSES // NC  # 125 classifier rows per core
EKC = INPUT_DIM // 128     # 16 k-chunks of the E contraction
SKC = S_DIM // 128         # 8 S chunks

# device block -> original 128-row chunk of the state vector
BLOCK_PERM = []
for c in range(NC):
    BLOCK_PERM += [c, 8 + 2 * c, 9 + 2 * c, 24 + c]

FILL_A = 280   # fillers between phase A(r+1) and phase B(r+1) (covers AG_b)
FILL_TAIL = 60  # fillers covering the O-gather before the classifier

_cache = {}


def _split_excess_waits(nc, mybir, limit=1, nop_limit=1):
    """This walrus build rejects multiple sync-waits on most instruction
    structs and any wait on Drain. Move excess waits onto preceding
    same-engine nops."""
    counter = [0]

    def make_nop(engine, waits):
        counter[0] += 1
        nop = mybir.InstNoOp(name=f"I-ws{counter[0]}", engine=engine)
        nop.sync_info = mybir.SyncInfo(on_wait=list(waits), on_update=[])
        return nop

    for fn in nc.m.functions:
        for bb in fn.blocks:
            out = []
            changed = False
            for inst in bb.instructions:
                si = getattr(inst, "sync_info", None)
                waits = list(si.on_wait) if si is not None and si.on_wait else []
                lim = 0 if isinstance(inst, mybir.InstDrain) else limit
                if len(waits) > lim:
                    keep = waits[-lim:] if lim else []
                    excess = waits[: len(waits) - lim]
                    for g in range(0, len(excess), nop_limit):
                        nop = make_nop(inst.engine, excess[g : g + nop_limit])
                        nc.register_instruction(nop, overwrite=True)
                        out.append(nop)
                    si.on_wait = keep
                    changed = True
                out.append(inst)
            if changed:
                bb.instructions = out


def _build(T):
    import concourse.bass as bass
    import concourse.tile as tile
    from concourse import mybir

    f32 = mybir.dt.float32
    bf16 = mybir.dt.bfloat16
    B = BATCH

    nc = bass.Bass()
    # all inputs arrive pre-tiled to the exact SBUF layout (partition-major)
    # so every load is a fully-contiguous DMA (~250GB/s vs ~25GB/s strided)
    wc = nc.dram_tensor("wc", [128, KC * 512], bf16, kind="ExternalInput")
    xT = nc.dram_tensor("xT", [128, EKC * B], bf16, kind="ExternalInput")
    inwT = nc.dram_tensor("inwT", [128, EKC * S_DIM], bf16, kind="ExternalInput")
    inbf = nc.dram_tensor("inbf", [128, SKC], f32, kind="ExternalInput")
    inwTc = nc.dram_tensor("inwTc", [128, EKC * 128], bf16, kind="ExternalInput")
    ws1 = nc.dram_tensor("ws1", [128, SKC * TOTAL], bf16, kind="ExternalInput")
    inbc = nc.dram_tensor("inbc", [128, 1], f32, kind="ExternalInput")
    outwTc = nc.dram_tensor("outwTc", [128, (O_DIM // 128) * CLS_M], bf16,
                            kind="ExternalInput")
    outbc = nc.dram_tensor("outbc", [CLS_M, 1], f32, kind="ExternalInput")
    ident = nc.dram_tensor("ident", [128, 128], bf16, kind="ExternalInput")
    out_t = nc.dram_tensor("out_t", [CLS_M, B], f32, kind="ExternalOutput")

    RG = [list(range(NC))]
    ROW_A = {k: 128 * (k // 4) for k in range(KC) if k % 4 == 0}
    ROW_B = {k: 384 * (k // 4) + 128 * (k % 4 - 1) for k in range(KC) if k % 4 >= 1}
    ACT_A = [k for k in range(KC) if k % 4 == 0]
    ACT_B = [k for k in range(KC) if k % 4 >= 1]

    def ag(ins_ap, out_ap):
        nc.gpsimd.collective_compute(
            "AllGather", mybir.AluOpType.bypass,
            replica_groups=RG, ins=[ins_ap], outs=[out_ap],
        )

    LOAD_ENGINES = [nc.sync, nc.scalar, nc.gpsimd]

    with tile.TileContext(nc) as tc:
        with (
            tc.tile_pool(name="wp", bufs=1) as wp,
            tc.tile_pool(name="pers", bufs=1) as pers,
            tc.tile_pool(name="state", bufs=1) as stp,
            tc.tile_pool(name="state2", bufs=1) as st2p,
            tc.tile_pool(name="res", bufs=2) as resp,
            tc.tile_pool(name="psum", bufs=1, space="PSUM") as psp,
            tc.tile_pool(name="psum1", bufs=1, space="PSUM") as psp1,
            tc.tile_pool(name="dram", bufs=2, space="DRAM") as dram,
        ):
            # --- static weights/consts into SBUF (contiguous DMAs) ---
            # W rows of the S chunks x ALL device columns (j-major so early
            # output chunks are consumable while later quarters stream in):
            # lets every core compute the FULL state_2 locally (state_1 is
            # replicated and sparse), so no collective is needed until state_3.
            ws1t = wp.tile([128, SKC * TOTAL], bf16, name="ws1t", tag="ws1t")
            QS = SKC * TOTAL // 4
            for q in range(4):
                nc.gpsimd.dma_start(ws1t[:, q * QS : (q + 1) * QS],
                                    ws1[:, q * QS : (q + 1) * QS])

            def ws1slice(sk, j):  # lhsT [128,128]: S-chunk sk rows, dev chunk j cols
                return ws1t[:, j * S_DIM + 128 * sk : j * S_DIM + 128 * (sk + 1)]

            # wt m-major: block-0 weights land first so round 2's m0 pass
            # (which feeds the first collective) is not gated on the full W
            wt = wp.tile([128, KC * 512], bf16, name="wt", tag="wt")
            QW4 = KC * 512 // 4
            for q in range(4):
                nc.sync.dma_start(wt[:, q * QW4 : (q + 1) * QW4],
                                  wc[:, q * QW4 : (q + 1) * QW4])

            def wslice(k, m):  # lhsT tile [128, 128] for (k-chunk, m-block)
                return wt[:, m * TOTAL + 128 * k : m * TOTAL + 128 * (k + 1)]

            xt = pers.tile([128, EKC * B], bf16, name="xt", tag="xt")
            # full in_w.T for the replicated E phase: tiles (k, mm)
            iwt = wp.tile([128, EKC * S_DIM], bf16, name="iwt", tag="iwt")
            # interleave xt/iwt k-range chunks so the E phase can start
            # consuming k=0 while later chunks are still in flight
            QX = EKC * B // 4
            QW = EKC * S_DIM // 4
            for q in range(4):
                nc.scalar.dma_start(xt[:, q * QX : (q + 1) * QX],
                                    xT[:, q * QX : (q + 1) * QX])
                nc.scalar.dma_start(iwt[:, q * QW : (q + 1) * QW],
                                    inwT[:, q * QW : (q + 1) * QW])
            # own-chunk in_w.T for einj
            iwtc = pers.tile([128, INPUT_DIM], bf16, name="iwtc", tag="iwtc")
            nc.gpsimd.dma_start(iwtc[:], inwTc[:])
            id_t = pers.tile([128, 128], bf16, name="ident", tag="ident")
            nc.gpsimd.dma_start(id_t[:], ident[:])
            inb_t = pers.tile([128, SKC], f32, name="inbf", tag="inbf")
            nc.gpsimd.dma_start(inb_t[:], inbf[:])
            inbc_t = pers.tile([128, 1], f32, name="inbc", tag="inbc")
            nc.gpsimd.dma_start(inbc_t[:], inbc[:])
            outb_t = pers.tile([CLS_M, 1], f32, name="outb", tag="outb")
            nc.gpsimd.dma_start(outb_t[:], outbc[:])
            owt = pers.tile([128, (O_DIM // 128) * CLS_M], bf16, name="owt", tag="owt")
            nc.gpsimd.dma_start(owt[:], outwTc[:])

            def iwslice(k, mm):
                return iwt[:, k * S_DIM + 128 * mm : k * S_DIM + 128 * (mm + 1)]

            # --- replicated E phase: st1[mm] = relu(E chunk mm) for all mm ---
            # k-outer across 8 open accumulation groups, one PSUM bank each
            # (start=True clears per-bank state, so concurrent groups must
            # not share a bank). k-outer lets E start as soon as xt and the
            # first iwt chunks land, well inside the cold-communicator window.
            px = psp1.tile([128, 512], f32, name="px", tag="px")
            ebank = {}
            for mm in range(4):
                ebank[mm] = psp.tile([128, B], f32, name=f"pse{mm}",
                                     tag=f"ps{mm}")[:]
            ebank[4] = psp1.tile([128, B], f32, name="pe4", tag="pe4")[:]
            ebank[5] = psp1.tile([128, B], f32, name="pe5", tag="pe5")[:]
            ebank[6] = psp1.tile([128, B], f32, name="pe6", tag="pe6")[:]
            ebank[7] = px[:, 0:256]
            st1 = {}
            for k in range(EKC):
                for mm in range(SKC):
                    nc.tensor.matmul(
                        ebank[mm], iwslice(k, mm), xt[:, B * k : B * (k + 1)],
                        start=(k == 0), stop=(k == EKC - 1),
                    )
            for mm in range(SKC):
                st1[mm] = resp.tile([128, B], bf16, name=f"st1_{mm}",
                                    tag=f"st1_{mm}")
                nc.scalar.activation(
                    st1[mm][:], ebank[mm], mybir.ActivationFunctionType.Relu,
                    bias=inb_t[:, mm : mm + 1],
                )
            ps_d = px[:, 256:384]
            einj = pers.tile([128, B], bf16, name="einj", tag="einj")

            def emit_einj():  # own-chunk E + bias (no relu), for t=5 injection
                ps_j = px[:, 0:256]
                for k in range(EKC):
                    nc.tensor.matmul(
                        ps_j, iwtc[:, 128 * k : 128 * (k + 1)],
                        xt[:, B * k : B * (k + 1)],
                        start=(k == 0), stop=(k == EKC - 1),
                    )
                nc.scalar.activation(
                    einj[:], ps_j, mybir.ActivationFunctionType.Identity,
                    bias=inbc_t[:],
                )

            def fill(n, rhs):  # keep PE busy/warm; rhs pins the fillers
                for _ in range(n):
                    nc.tensor.matmul(ps_d, id_t[:], rhs[:, 0:128],
                                     start=True, stop=True)

            # --- rounds: state_{r+1} = relu(W.T @ state_r [+ inj]) ---
            # Round 1 computes the FULL state_2 on every core (state_1 is
            # replicated and only its S rows are nonzero, so the contraction
            # is over 8 chunks using ws1t) -- no collective until state_3.
            last = T - 1
            res = {}
            agout_a = agout_b = None
            prev_a_loaded = {}
            st2 = {}
            EB_TAGS = ["ps0", "ps1", "ps2", "ps3", "pe4", "pe5", "pe6", None]

            def ebank_acc(j):
                tag = EB_TAGS[j % 8]
                if tag is None:
                    return px[:, 0:256]
                pool = psp if tag.startswith("ps") else psp1
                return pool.tile([128, B], f32, name=f"sb{j}", tag=tag)[:]

            if T == 2:
                ps3 = psp.tile([128, B], f32, name="ps3", tag="ps3")[:]
                for i in range(SKC):
                    nc.tensor.matmul(
                        ps3, wslice(4 * i, MPC - 1), st1[i][:],
                        start=(i == 0), stop=(i == SKC - 1),
                    )
                res = {MPC - 1: resp.tile([128, B], bf16, name="res3", tag="res3")}
                nc.vector.tensor_relu(res[MPC - 1][:], ps3)
            else:
                # round 1: full local state_2, 32 chunks x 8 S-chunk contraction
                for j in range(KC):
                    acc = ebank_acc(j)
                    for sk in range(SKC):
                        nc.tensor.matmul(
                            acc, ws1slice(sk, j), st1[sk][:],
                            start=(sk == 0), stop=(sk == SKC - 1),
                        )
                    st2[j] = st2p.tile([128, B], bf16, name=f"st2_{j}",
                                       tag=f"st2_{j}")
                    nc.vector.tensor_relu(st2[j][:], acc)
                if T >= 7:
                    emit_einj()  # used by round 5; runs in the post-r2 gap

                for r in range(2, T):
                    inject = r % 5 == 0
                    m_list = [MPC - 1] if r == last else list(range(MPC))
                    ps = {}
                    for m in m_list:
                        t_ = psp.tile([128, B], f32, name=f"ps{m}", tag=f"ps{m}")
                        ps[m] = t_[:]
                    res = {}
                    early = [0] if (0 in m_list and r != last) else []
                    rest = [m for m in m_list if m not in early]

                    if r == 2:
                        # local contraction over st2 -- no loads, no waits
                        def chunk_src(k):
                            return st2[k][:]
                        order = list(range(KC))
                        a_part = []
                        b_part = order
                    else:
                        # fillers first (pinned to freshly-loaded a-chunks),
                        # then phase A right before phase B: no PE idle gap
                        # >3us, so the HAM clock gate stays warm into phase B
                        fill(FILL_A, rhs=prev_a_loaded[ACT_A[0]])
                        st = {}
                        SPANS = [(0, 9), (9, 9), (18, 6)]  # (chunk0, nchunks)
                        for q, (c0, nch) in enumerate(SPANS):
                            gt = stp.tile([128, nch * B], bf16, name=f"sp{q}",
                                          tag=f"sp{q}")
                            LOAD_ENGINES[q].dma_start(
                                gt[:].rearrange("p (k b) -> p k b", b=B),
                                agout_b[128 * c0 : 128 * (c0 + nch), :]
                                .rearrange("(k p) b -> p k b", p=128),
                            )
                            for j in range(nch):
                                ci = c0 + j  # b-chunk index 0..23
                                g, jj = divmod(ci, 3)
                                st[4 * g + 1 + jj] = gt[:, j * B : (j + 1) * B]

                        def chunk_src(k):
                            return (prev_a_loaded[k][:] if k % 4 == 0
                                    else st[k])
                        a_part = ACT_A
                        b_part = ACT_B

                    # phase A (empty for r==2)
                    for i, k in enumerate(a_part):
                        for m in m_list:
                            nc.tensor.matmul(
                                ps[m], wslice(k, m), chunk_src(k),
                                start=(i == 0), stop=False,
                            )
                    # m0 first over the remaining chunks, then AG_a launch
                    for m in early:
                        for j, k in enumerate(b_part):
                            nc.tensor.matmul(
                                ps[m], wslice(k, m), chunk_src(k),
                                start=(not a_part) and j == 0,
                                stop=(j == len(b_part) - 1) and not inject,
                            )
                        if inject:
                            nc.tensor.matmul(ps[0], id_t[:], einj[:],
                                             start=False, stop=True)
                        res[0] = resp.tile([128, B], bf16, name="res0",
                                           tag="res0")
                        nc.scalar.activation(
                            res[0][:], ps[0], mybir.ActivationFunctionType.Relu
                        )
                        new_a = dram.tile([128, B], bf16, name="aga", tag="aga")
                        nc.scalar.dma_start(new_a[0:128, :], res[0][:])
                        agout_a = dram.tile(
                            [1024, B], bf16, name="agouta", tag="agouta",
                            addr_space="Shared"
                        )
                        ag(new_a.opt(), agout_a.opt())
                    # remaining blocks
                    for j, k in enumerate(b_part):
                        for m in rest:
                            nc.tensor.matmul(
                                ps[m], wslice(k, m), chunk_src(k),
                                start=(not a_part) and j == 0,
                                stop=(j == len(b_part) - 1),
                            )
                    for m in rest:
                        res[m] = resp.tile([128, B], bf16, name=f"res{m}",
                                           tag=f"res{m}")
                        nc.vector.tensor_relu(res[m][:], ps[m])
                    if r == last:
                        break
                    new_b = dram.tile([384, B], bf16, name="agb", tag="agb")
                    nc.sync.dma_start(new_b[0:128, :], res[1][:])
                    nc.scalar.dma_start(new_b[128:256, :], res[2][:])
                    nc.gpsimd.dma_start(new_b[256:384, :], res[3][:])
                    agout_b = dram.tile(
                        [3072, B], bf16, name="agoutb", tag="agoutb",
                        addr_space="Shared"
                    )
                    ag(new_b.opt(), agout_b.opt())
                    # pre-load next round's phase-A chunks (arrive mid-flight)
                    prev_a_loaded = {}
                    for i, k in enumerate(ACT_A):
                        t_ = stp.tile([128, B], bf16, name=f"sa{k}", tag=f"sa{k}")
                        LOAD_ENGINES[i % 3].dma_start(
                            t_[:], agout_a[ROW_A[k] : ROW_A[k] + 128, :]
                        )
                        prev_a_loaded[k] = t_

            # --- gather O chunks, classifier slice, bias, out ---
            ago_in = dram.tile([128, B], bf16, name="agoin", tag="agoin")
            nc.sync.dma_start(ago_in[:], res[MPC - 1][:])
            ago_out = dram.tile(
                [O_DIM, B], bf16, name="agoout", tag="agoout", addr_space="Shared"
            )
            ag(ago_in.opt(), ago_out.opt())
            fill(FILL_TAIL, rhs=res[MPC - 1])
            ot = {}
            for k in range(O_DIM // 128):
                ot[k] = stp.tile([128, B], bf16, name=f"ot{k}", tag=f"st{4 * (k % 6) + 1}")
                LOAD_ENGINES[k % 3].dma_start(
                    ot[k][:], ago_out[128 * k : 128 * (k + 1), :]
                )
            ps_c = psp1.tile([CLS_M, B], f32, name="psc", tag="pe4")
            for k in range(O_DIM // 128):
                nc.tensor.matmul(
                    ps_c[:], owt[:, CLS_M * k : CLS_M * (k + 1)], ot[k][:],
                    start=(k == 0), stop=(k == O_DIM // 128 - 1),
                )
            out_sb = pers.tile([CLS_M, B], f32, name="outsb", tag="outsb")
            nc.scalar.activation(
                out_sb[:], ps_c[:], mybir.ActivationFunctionType.Identity,
                bias=outb_t[:],
            )
            nc.sync.dma_start(out_t[:], out_sb[:])

    _split_excess_waits(nc, mybir)
    return nc


def kernel(x, W, in_w, in_b, out_w, out_b, time_steps):
    T = int(time_steps)
    x = np.ascontiguousarray(x, dtype=np.float32)
    W = np.ascontiguousarray(W, dtype=np.float32)
    in_w = np.ascontiguousarray(in_w, dtype=np.float32)
    in_b = np.ascontiguousarray(in_b, dtype=np.float32)
    out_w = np.ascontiguousarray(out_w, dtype=np.float32)
    out_b = np.ascontiguousarray(out_b, dtype=np.float32)

    if T < 2:
        # T=0: O stays 0; T=1: state_1 = [relu(E),0,0], O still 0.
        return np.broadcast_to(out_b, (BATCH, NUM_CLASSES)).astype(np.float32).copy()

    import ml_dtypes
    from concourse.bass_utils import run_bass_kernel_spmd

    if T not in _cache:
        _cache[T] = _build(T)
    nc = _cache[T]

    bf = ml_dtypes.bfloat16

    def sbuf_tile(arr2d):  # [K*128, M] -> partition-major [128, K*M]
        K = arr2d.shape[0] // 128
        return np.ascontiguousarray(
            arr2d.reshape(K, 128, -1).transpose(1, 0, 2).reshape(128, -1)
        )

    # block-permute W rows and columns to the device layout
    Wd = W.reshape(KC, 128, TOTAL)[BLOCK_PERM].reshape(TOTAL, TOTAL)
    Wd = Wd.reshape(TOTAL, KC, 128)[:, BLOCK_PERM].reshape(TOTAL, TOTAL)
    Wd = Wd.astype(bf)
    xTa = sbuf_tile(x.T.astype(bf))
    inwTa = in_w.T.astype(bf)
    inwT_t = sbuf_tile(inwTa)
    outwT = out_w.T.astype(bf)
    ident = np.eye(128, dtype=np.float32).astype(bf)
    inb_full = np.ascontiguousarray(in_b.reshape(SKC, 128).T.astype(np.float32))

    ws1_host = np.ascontiguousarray(
        Wd.reshape(KC, 128, TOTAL)[[4 * sk for sk in range(SKC)]]
        .reshape(SKC, 128, KC, 128).transpose(1, 2, 0, 3)
        .reshape(128, SKC * TOTAL)
    )
    in_maps = []
    for c in range(NC):
        in_maps.append({
            "wc": np.ascontiguousarray(
                Wd[:, 512 * c : 512 * (c + 1)]
                .reshape(KC, 128, MPC, 128).transpose(1, 2, 0, 3)
                .reshape(128, KC * 512)
            ),
            "ws1": ws1_host,
            "xT": xTa,
            "inwT": inwT_t,
            "inbf": inb_full,
            "inwTc": sbuf_tile(
                np.ascontiguousarray(inwTa[:, 128 * c : 128 * (c + 1)])
            ),
            "inbc": np.ascontiguousarray(in_b[128 * c : 128 * (c + 1), None]),
            "outwTc": sbuf_tile(
                np.ascontiguousarray(outwT[:, CLS_M * c : CLS_M * (c + 1)])
            ),
            "outbc": np.ascontiguousarray(out_b[CLS_M * c : CLS_M * (c + 1), None]),
            "ident": ident,
        })
    res = run_bass_kernel_spmd(nc, in_maps, list(range(NC)))
    outT = np.concatenate([res.results[c]["out_t"] for c in range(NC)], axis=0)
    return np.ascontiguousarray(outT.T)
